# revision 46
# baseline (speedup 1.0000x reference)
"""Trainium2 Bass kernel for nn_L2MLoRAqkv (MoE-routed LoRA QKV projection).

Math (per batch b, expert i = idx[b,0]):
    qkv = x @ W.T + bias
    qkv[:, :D]  += (x @ A_q[i]) @ B_q[i] * SCALE
    qkv[:, -D:] += (x @ A_v[i]) @ B_v[i] * SCALE

Strategy: data-parallel over the batch dim (1 batch per NeuronCore, 8 cores).
On the host we gather each batch's expert and fold the rank-8 LoRA update
into the (transposed) projection weight in float64:
    W_eff[b] = W.T; W_eff[:, :D] += A_q[i] @ B_q[i]; W_eff[:, -D:] += A_v[i] @ B_v[i]
so the device kernel is a single dense GEMM per core:
    Y[4096, 3072] = X[4096, 1024] @ W_eff[1024, 3072] + bias
X is pre-transposed on the host ([D, T], K-major) so both matmul operands
load with K on SBUF partitions via fast contiguous DMAs.
"""

import os
import sys

import numpy as np

for _p in ("/opt/trn_rl_repo",):
    if _p not in sys.path and os.path.isdir(_p):
        sys.path.insert(0, _p)

B = 8          # batches == cores
T = 4096       # tokens per batch
D = 1024       # model dim (contraction K)
N3 = 3072      # qkv output dim
P = 128        # SBUF partitions
NT = 512       # n-tile (one fp32 PSUM bank)
CHUNK = 512    # token chunk streamed per DMA group
KT = D // P        # 8 k-tiles
NN = N3 // NT      # 6 n-tiles
TT = CHUNK // P    # 4 token sub-tiles per chunk
SCALE = 8.0 / 8.0

MM_DTYPE = "float16"  # 1 cycle/row on PE (same as f32r) at half the DMA bytes

_NC_CACHE = {}


def _build(mm_dtype_name=MM_DTYPE, tokens=T):
    import concourse.tile as tile
    from concourse import bacc, mybir

    nchunk = tokens // CHUNK
    mmdt = getattr(mybir.dt, mm_dtype_name)
    f32 = mybir.dt.float32

    nc = bacc.Bacc(
        "TRN2",
        target_bir_lowering=False,
        debug=False,
        enable_asserts=False,
        num_devices=B,
    )
    f16 = mybir.dt.float16
    xt = nc.dram_tensor("xt", [D, tokens], mmdt, kind="ExternalInput").ap()
    weff = nc.dram_tensor("weff", [D, N3], mmdt, kind="ExternalInput").ap()
    biasr = nc.dram_tensor("biasr", [1, N3], f16, kind="ExternalInput").ap()
    y = nc.dram_tensor("y", [tokens, N3], f16, kind="ExternalOutput").ap()

    with tile.TileContext(nc) as tc:
        with tc.tile_pool(name="const", bufs=1) as const_pool, \
             tc.tile_pool(name="xin", bufs=3) as xin_pool, \
             tc.tile_pool(name="outp", bufs=4) as out_pool, \
             tc.tile_pool(name="outh", bufs=TT * NN) as outh_pool, \
             tc.tile_pool(name="ps", bufs=8, space="PSUM") as psum_pool:

            def load_chunk(c, split=False):
                # X.T chunk: 8 k-tiles of [128, CHUNK] side by side.
                xc = xin_pool.tile([P, KT * CHUNK], mmdt, tag="xc", name="xc")
                for k in range(KT):
                    # TRN2 has two HWDGE rings (sync + scalar); split the
                    # startup-critical chunk across both.
                    eng = nc.scalar if (split and k % 2) else nc.sync
                    eng.dma_start(
                        xc[:, k * CHUNK : (k + 1) * CHUNK],
                        xt[k * P : (k + 1) * P, c * CHUNK : (c + 1) * CHUNK],
                    )
                return xc

            # PE warmup: ~36 dummy matmuls on zeroed SBUF ramp the PE
            # p-state (0.65 -> 2.4 GHz needs ~3 us of continuous work)
            # during the DMA startup window, so the first real groups run
            # at full clock.
            wz = const_pool.tile([P, NT], f16)
            nc.vector.memset(wz[:], 0.0)
            wps = psum_pool.tile([P, NT], f32, tag="ps", name="ps")
            for _ in range(30):
                nc.tensor.matmul(
                    wps[:], lhsT=wz[:, 0:P], rhs=wz[:], start=True, stop=True
                )

            # Chunk 0 of X first: the first matmul group needs it, and the
            # DGE rings drain in issue order.  bias goes via SWDGE.
            head = 1
            # bias: one 6 KB row from HBM, broadcast on-chip — keeps 0.75 MB
            # out of the bandwidth-contended startup window.
            xcs_head = [load_chunk(0, split=True)]
            bias_row = const_pool.tile([1, N3], f16)
            nc.gpsimd.dma_start(bias_row[:], biasr[:])
            bias_sb = const_pool.tile([P, N3], f16)
            nc.gpsimd.partition_broadcast(bias_sb[:], bias_row[:])

            # W_eff resident in SBUF as 8 k-slices side by side: [128, 8*3072].
            # DMA n-slice-major so the first matmul groups unblock early,
            # split across both HWDGE rings for bandwidth.
            w_sb = const_pool.tile([P, KT * N3], mmdt)
            for n in range(NN):
                for k in range(KT):
                    eng = nc.scalar if k % 2 else nc.sync
                    dst = w_sb[:, k * N3 + n * NT : k * N3 + (n + 1) * NT]
                    src = weff[k * P : (k + 1) * P, n * NT : (n + 1) * NT]
                    if n == 1:
                        # n1 is the first slice the PE waits on after the
                        # startup set; partition halves halve its per-queue
                        # latency.
                        h = P // 2
                        eng.dma_start(dst[0:h, :], src[0:h, :])
                        eng.dma_start(dst[h:P, :], src[h:P, :])
                    else:
                        eng.dma_start(dst, src)

            store_ctr = [0]

            def store(ob, c, t, n, ways=1):
                # Stores alternate between the two HWDGE rings; the final
                # groups split 4 ways to trim the end-of-kernel latency.
                row = c * CHUNK + t * P
                h = P // ways
                for w in range(ways):
                    eng = nc.scalar if store_ctr[0] % 2 else nc.sync
                    store_ctr[0] += 1
                    eng.dma_start(
                        y[row + w * h : row + (w + 1) * h, n * NT : (n + 1) * NT],
                        ob[w * h : (w + 1) * h, :],
                    )

            def drain(ps, c, t, n, ways=1, defer=None):
                # Head-chunk stores are deferred: their dma_starts would
                # otherwise compete with the W n4/n5 loads the PE is pacing
                # on during the startup window.
                pool = outh_pool if defer is not None else out_pool
                tag = "obh" if defer is not None else "ob"
                ob = pool.tile([P, NT], f16, tag=tag, name=tag)
                nc.vector.tensor_add(ob[:], ps[:], bias_sb[:, n * NT : (n + 1) * NT])
                if defer is not None:
                    defer.append((ob, c, t, n))
                else:
                    store(ob, c, t, n, ways)

            def do_group(xc, c, t, n, ways=1, defer=None):
                ps = psum_pool.tile([P, NT], f32, tag="ps", name="ps")
                for k in range(KT):
                    nc.tensor.matmul(
                        ps[:],
                        lhsT=xc[:, k * CHUNK + t * P : k * CHUNK + (t + 1) * P],
                        rhs=w_sb[:, k * N3 + n * NT : k * N3 + (n + 1) * NT],
                        start=(k == 0),
                        stop=(k == KT - 1),
                    )
                drain(ps, c, t, n, ways, defer)

            # Head chunk n-outer, so matmul groups unblock in weff
            # DMA-arrival order and never outrun the loads.  Stores deferred
            # out of the W-load window.
            deferred = []
            for n in range(NN):
                for c in range(head):
                    for t in range(TT):
                        do_group(xcs_head[c], c, t, n, defer=deferred)

            # Remaining chunks: weff fully resident.  Chunk c+0's loads are
            # emitted before the deferred head stores so they lead them in
            # ring program order.
            for c in range(head, nchunk):
                xc = load_chunk(c)
                if deferred:
                    for ob, dc, dt, dn in deferred:
                        store(ob, dc, dt, dn)
                    deferred = []
                last = c == nchunk - 1
                for t in range(TT):
                    for n in range(NN):
                        ways = 8 if (last and t == TT - 1) else 1
                        do_group(xc, c, t, n, ways)
    nc.compile()
    return nc


def _get_nc(mm_dtype_name=MM_DTYPE, tokens=T):
    key = (mm_dtype_name, tokens)
    if key not in _NC_CACHE:
        _NC_CACHE[key] = _build(mm_dtype_name, tokens)
    return _NC_CACHE[key]


def _prep_in_maps(inputs):
    x = np.asarray(inputs["x"], dtype=np.float32)
    weight = np.asarray(inputs["weight"], dtype=np.float32)
    bias = np.asarray(inputs["bias"], dtype=np.float32)
    aq = np.asarray(inputs["A_q_pool"], dtype=np.float32)
    bq = np.asarray(inputs["B_q_pool"], dtype=np.float32)
    av = np.asarray(inputs["A_v_pool"], dtype=np.float32)
    bv = np.asarray(inputs["B_v_pool"], dtype=np.float32)
    idx = np.asarray(inputs["idx"]).reshape(B, -1)[:, 0].astype(np.int64)

    wt64 = weight.T.astype(np.float64)  # [D, N3]
    biasr = np.ascontiguousarray(bias.astype(np.float16)[None, :])
    xts = x.transpose(0, 2, 1)  # [B, D, T] strided view

    in_maps = []
    for b in range(B):
        i = int(idx[b])
        weff = wt64.copy()
        weff[:, :D] += SCALE * (aq[i].astype(np.float64) @ bq[i].astype(np.float64))
        weff[:, N3 - D:] += SCALE * (av[i].astype(np.float64) @ bv[i].astype(np.float64))
        in_maps.append({
            "xt": np.ascontiguousarray(xts[b]).astype(np.float16),
            "weff": weff.astype(np.float16),
            "biasr": biasr,
        })
    return in_maps


def _run(in_maps, trace=False, **kwargs):
    from concourse.bass_utils import run_bass_kernel_spmd

    nc = _get_nc()
    return run_bass_kernel_spmd(
        nc, in_maps, core_ids=list(range(B)), trace=trace, **kwargs
    )


def kernel(**inputs):
    res = _run(_prep_in_maps(inputs), trace=False)
    return np.stack([r["y"].astype(np.float32) for r in res.results], axis=0)

